# revision 12
# baseline (speedup 1.0000x reference)
"""Memory-Network kernel for 8 Trainium2 NeuronCores.

Data-parallel: batch B=128 is split 16-per-core; each core processes its
160 (b, r) sequences end-to-end (embedding gather, q/f LSTMs, attention,
FC) with no collectives. Weights are replicated; all layout prep
(transposes, gate permutation, bf16 casts, padding) happens on host.

Device layout convention: everything feature-major [feature, token] so
the LSTM recurrence's matmuls keep gates/hidden on the partition dim.
"""

import sys

for _p in ("/opt/trn_rl_repo", "/root/.axon_site/_ro/trn_rl_repo"):
    if _p not in sys.path:
        sys.path.insert(0, _p)

import numpy as np
import ml_dtypes

import concourse.bass as bass
import concourse.mybir as mybir
import concourse.tile as tile
from concourse import bacc
from concourse.bass_utils import run_bass_kernel_spmd
from concourse.masks import make_identity

BF16 = mybir.dt.bfloat16
F32 = mybir.dt.float32
I32 = mybir.dt.int32

NP_BF16 = ml_dtypes.bfloat16

VOCAB, EMB, HID, IMG = 50000, 300, 512, 4096
B, R, LQ, LH = 128, 10, 20, 40
N_CORES = 8
BS = B // N_CORES          # 16 batch items per core
S = BS * R                 # 160 sequences per core
EPAD = 384                 # embedding row padded to 3x128 for K-chunking
G4 = 4 * HID               # 2048 gate rows
KQ = EPAD + HID            # 896 fused contraction dim (x + h)
NMT = G4 // 128            # 16 gate m-tiles
NEG = -1.0e30

_STATE = None
_DEBUG = False


def _gate_perm():
    # m-tile m = 4*j + gate: hidden chunk j's (i, f, g, o) slices adjacent
    return np.concatenate(
        [np.arange(g * HID + j * 128, g * HID + (j + 1) * 128)
         for j in range(4) for g in range(4)]
    )


def _build_program():
    nc = bacc.Bacc()

    dt_in = {}

    def din(name, shape, dtype):
        dt_in[name] = nc.dram_tensor(name, list(shape), dtype, kind="ExternalInput")
        return dt_in[name]

    emb_d = din("embp", [VOCAB, EPAD], BF16)
    idxq_d = din("idxq", [128, LQ * S // 128], I32)    # [128, 25]
    idxf_d = din("idxf", [128, LH * S // 128], I32)    # [128, 50]
    wq_d = din("wq", [KQ, G4], BF16)
    wf_d = din("wf", [KQ, G4], BF16)
    bq_d = din("bq", [128, NMT], F32)
    bf_d = din("bf", [128, NMT], F32)
    w1i_d = din("w1i", [IMG, HID], BF16)
    w1h_d = din("w1h", [HID, HID], BF16)
    b1_d = din("b1", [128, 4], F32)
    w2_d = din("w2", [HID, HID], BF16)
    b2_d = din("b2", [128, 4], F32)
    img_d = din("imgrep", [IMG, S], BF16)
    mask_d = din("mask", [S, S], F32)
    out_d = nc.dram_tensor("out", [HID, S], F32, kind="ExternalOutput")
    dbg = {}
    if _DEBUG:
        for c in range(3):
            dbg[f"eq{c}"] = nc.dram_tensor(
                f"dbg_eq{c}", [128, LQ * S], BF16, kind="ExternalOutput")
        dbg["hq"] = nc.dram_tensor("dbg_hq", [HID, S], BF16, kind="ExternalOutput")
        dbg["hf"] = nc.dram_tensor("dbg_hf", [HID, S], BF16, kind="ExternalOutput")
        dbg["qt"] = nc.dram_tensor("dbg_qt", [HID, S], F32, kind="ExternalOutput")
        dbg["att"] = nc.dram_tensor("dbg_att", [HID, S], BF16, kind="ExternalOutput")
        dbg["abf"] = nc.dram_tensor("dbg_abf", [S, S], BF16, kind="ExternalOutput")

    NBQ = LQ * S // 128   # 25 gather blocks for questions
    NBF = LH * S // 128   # 50 gather blocks for history

    with tile.TileContext(nc) as tc:
        with (
            tc.tile_pool(name="consts", bufs=1) as cp,
            tc.tile_pool(name="gather", bufs=6) as gp,
            tc.tile_pool(name="hstate", bufs=12) as hp,
            tc.tile_pool(name="cstate", bufs=8) as cpool,
            tc.tile_pool(name="ew", bufs=20) as ew,
            tc.tile_pool(name="w1s", bufs=2) as w1p,
            tc.tile_pool(name="outp", bufs=4) as op,
            tc.tile_pool(name="ps", bufs=8, space="PSUM") as ps,
        ):
            # ---------- phase 0: constants in, gathers, transposes ----------
            idxq = cp.tile([128, NBQ], I32, name="idxq_sb", tag="idxq")
            nc.scalar.dma_start(idxq[:], idxq_d.ap()[:])
            idxf = cp.tile([128, NBF], I32, name="idxf_sb", tag="idxf")
            nc.scalar.dma_start(idxf[:], idxf_d.ap()[:])

            wq_sb = cp.tile([128, 7, G4], BF16, name="wq_sb", tag="wq")
            nc.scalar.dma_start(
                wq_sb[:], wq_d.ap().rearrange("(k p) m -> p k m", p=128))
            wf_sb = cp.tile([128, 7, G4], BF16, name="wf_sb", tag="wf")
            nc.scalar.dma_start(
                wf_sb[:], wf_d.ap().rearrange("(k p) m -> p k m", p=128))
            bq_sb = cp.tile([128, NMT], F32, name="bq_sb", tag="bq")
            nc.scalar.dma_start(bq_sb[:], bq_d.ap()[:])
            bf_sb = cp.tile([128, NMT], F32, name="bf_sb", tag="bf")
            nc.scalar.dma_start(bf_sb[:], bf_d.ap()[:])

            w1h_sb = cp.tile([128, 4, HID], BF16, name="w1h_sb", tag="w1h")
            nc.scalar.dma_start(
                w1h_sb[:], w1h_d.ap().rearrange("(k p) m -> p k m", p=128))
            w2_sb = cp.tile([128, 4, HID], BF16, name="w2_sb", tag="w2")
            nc.scalar.dma_start(
                w2_sb[:], w2_d.ap().rearrange("(k p) m -> p k m", p=128))
            b1_sb = cp.tile([128, 4], F32, name="b1_sb", tag="b1")
            nc.scalar.dma_start(b1_sb[:], b1_d.ap()[:])
            b2_sb = cp.tile([128, 4], F32, name="b2_sb", tag="b2")
            nc.scalar.dma_start(b2_sb[:], b2_d.ap()[:])
            img_sb = cp.tile([128, IMG // 128, S], BF16, name="img_sb", tag="img")
            nc.scalar.dma_start(
                img_sb[:], img_d.ap().rearrange("(k p) m -> p k m", p=128))
            mask_sb = cp.tile([128, 2, S], F32, name="mask_sb", tag="mask")
            nc.scalar.dma_start(
                mask_sb[:, 0, :], mask_d.ap()[0:128, :])
            nc.scalar.dma_start(
                mask_sb[0:S - 128, 1, :], mask_d.ap()[128:S, :])

            ident = cp.tile([128, 128], BF16, name="ident", tag="ident")
            make_identity(nc, ident[:])

            # gathered embeddings, feature-major: 3 chunks of 128 features
            eq = [cp.tile([128, LQ * S], BF16, name=f"eq{c}", tag=f"eq{c}")
                  for c in range(3)]
            ef = [cp.tile([128, LH * S], BF16, name=f"ef{c}", tag=f"ef{c}")
                  for c in range(3)]

            def gather_blocks(idx_sb, nblk, dst):
                for g in range(nblk):
                    gt = gp.tile([128, EPAD], BF16, name="gt", tag="gt")
                    nc.gpsimd.indirect_dma_start(
                        out=gt[:],
                        out_offset=None,
                        in_=emb_d.ap()[:],
                        in_offset=bass.IndirectOffsetOnAxis(
                            ap=idx_sb[:, g:g + 1], axis=0),
                    )
                    for c in range(3):
                        nc.sync.dma_start_transpose(
                            dst[c][:, g * 128:(g + 1) * 128],
                            gt[:, c * 128:(c + 1) * 128])

            gather_blocks(idxq, NBQ, eq)
            gather_blocks(idxf, NBF, ef)
            if _DEBUG:
                for c in range(3):
                    nc.sync.dma_start(dbg[f"eq{c}"].ap()[:], eq[c][:])

            # ---------- LSTM recurrence ----------
            def lstm(T, e_chunks, w_sb, b_sb, label):
                h = None
                c_st = None
                for t in range(T):
                    rhs_list = [e_chunks[c][:, t * S:(t + 1) * S] for c in range(3)]
                    nk = 3
                    if t > 0:
                        rhs_list += h
                        nk = 7
                    new_h, new_c = [], []
                    for j in range(4):
                        # one PSUM bank per gate m-tile; chunk j = m 4j..4j+3
                        pg = [ps.tile([128, S], F32, name=f"pg{label}", tag="pg")
                              for _ in range(4)]
                        for ki in range(nk):
                            for g in range(4):
                                m = 4 * j + g
                                nc.tensor.matmul(
                                    pg[g][:],
                                    lhsT=w_sb[:, ki, m * 128:(m + 1) * 128],
                                    rhs=rhs_list[ki],
                                    start=(ki == 0),
                                    stop=(ki == nk - 1),
                                )
                        mi, mf_, mg, mo = 4 * j, 4 * j + 1, 4 * j + 2, 4 * j + 3
                        si = ew.tile([128, S], F32, name="si", tag="ew")
                        nc.scalar.activation(
                            si[:], pg[0][:], mybir.ActivationFunctionType.Sigmoid,
                            bias=b_sb[:, mi:mi + 1])
                        tg = ew.tile([128, S], F32, name="tg", tag="ew")
                        nc.scalar.activation(
                            tg[:], pg[2][:], mybir.ActivationFunctionType.Tanh,
                            bias=b_sb[:, mg:mg + 1])
                        so = ew.tile([128, S], F32, name="so", tag="ew")
                        nc.scalar.activation(
                            so[:], pg[3][:], mybir.ActivationFunctionType.Sigmoid,
                            bias=b_sb[:, mo:mo + 1])
                        cn = cpool.tile([128, S], F32, name="cn", tag="c")
                        if t == 0:
                            nc.vector.tensor_mul(cn[:], si[:], tg[:])
                        else:
                            sf = ew.tile([128, S], F32, name="sf", tag="ew")
                            nc.scalar.activation(
                                sf[:], pg[1][:],
                                mybir.ActivationFunctionType.Sigmoid,
                                bias=b_sb[:, mf_:mf_ + 1])
                            m1 = ew.tile([128, S], F32, name="m1", tag="ew")
                            nc.vector.tensor_mul(m1[:], sf[:], c_st[j][:])
                            m2 = ew.tile([128, S], F32, name="m2", tag="ew")
                            nc.vector.tensor_mul(m2[:], si[:], tg[:])
                            nc.vector.tensor_add(cn[:], m1[:], m2[:])
                        tc_ = ew.tile([128, S], F32, name="tc", tag="ew")
                        nc.scalar.activation(
                            tc_[:], cn[:], mybir.ActivationFunctionType.Tanh)
                        hn = hp.tile([128, S], BF16, name="hn", tag="h")
                        nc.vector.tensor_mul(hn[:], so[:], tc_[:])
                        new_h.append(hn)
                        new_c.append(cn)
                    h, c_st = new_h, new_c
                return h

            hq = lstm(LQ, eq, wq_sb, bq_sb, "q")
            if _DEBUG:
                for j in range(4):
                    nc.sync.dma_start(
                        dbg["hq"].ap()[j * 128:(j + 1) * 128, :], hq[j][:])

            # ---------- query = tanh([img, hq] @ W1.T + b1) ----------
            pq = [ps.tile([128, S], F32, name="pq", tag="pg") for _ in range(4)]

            def qslice(m):
                return pq[m][:]

            n_im_blk = IMG // 1024  # 4 streamed lhsT blocks of 8 k-chunks
            for bI in range(n_im_blk):
                w1c = w1p.tile([128, 8, HID], BF16, name="w1c", tag="w1c")
                nc.scalar.dma_start(
                    w1c[:],
                    w1i_d.ap()[bI * 1024:(bI + 1) * 1024, :].rearrange(
                        "(k p) m -> p k m", p=128))
                for k8 in range(8):
                    ki = bI * 8 + k8
                    for m in range(4):
                        nc.tensor.matmul(
                            qslice(m),
                            lhsT=w1c[:, k8, m * 128:(m + 1) * 128],
                            rhs=img_sb[:, ki, :],
                            start=(ki == 0),
                            stop=False,
                        )
            for k in range(4):
                for m in range(4):
                    nc.tensor.matmul(
                        qslice(m),
                        lhsT=w1h_sb[:, k, m * 128:(m + 1) * 128],
                        rhs=hq[k][:],
                        start=False,
                        stop=(k == 3),
                    )
            qt_f = []
            qt_b = []
            for m in range(4):
                qf = cp.tile([128, S], F32, name=f"qtf{m}", tag=f"qtf{m}")
                nc.scalar.activation(
                    qf[:], qslice(m), mybir.ActivationFunctionType.Tanh,
                    bias=b1_sb[:, m:m + 1])
                qb = cp.tile([128, S], BF16, name=f"qtb{m}", tag=f"qtb{m}")
                nc.vector.tensor_copy(qb[:], qf[:])
                qt_f.append(qf)
                qt_b.append(qb)

            # ---------- fact LSTM ----------
            hf = lstm(LH, ef, wf_sb, bf_sb, "f")
            if _DEBUG:
                for j in range(4):
                    nc.sync.dma_start(
                        dbg["hf"].ap()[j * 128:(j + 1) * 128, :], hf[j][:])
                    nc.sync.dma_start(
                        dbg["qt"].ap()[j * 128:(j + 1) * 128, :], qt_f[j][:])

            # ---------- attention ----------
            # scores[n, n'] = sum_h Q[h, n] hf[h, n']  (2 partition tiles of n)
            sc = [ps.tile([128, S], F32, name="sc", tag="pg") for _ in range(2)]
            sc0, sc1 = sc[0][:, 0:S], sc[1][0:S - 128, 0:S]
            for k in range(4):
                nc.tensor.matmul(sc0, lhsT=qt_b[k][:, 0:128], rhs=hf[k][:],
                                 start=(k == 0), stop=(k == 3))
            for k in range(4):
                nc.tensor.matmul(sc1, lhsT=qt_b[k][:, 128:S], rhs=hf[k][:],
                                 start=(k == 0), stop=(k == 3))

            a_bf = []  # attention weights, 2 partition tiles [*, S] bf16
            for ti, (scp, npart) in enumerate([(sc0, 128), (sc1, S - 128)]):
                sm = ew.tile([128, S], F32, name="sm", tag="ew")
                nc.vector.tensor_add(sm[:npart], scp, mask_sb[:npart, ti, :])
                nmx = ew.tile([128, 1], F32, name="nmx", tag="red")
                nc.vector.tensor_reduce(
                    nmx[:npart], sm[:npart], mybir.AxisListType.X,
                    mybir.AluOpType.max, negate=True)
                ex = ew.tile([128, S], F32, name="ex", tag="ew")
                nc.scalar.activation(
                    ex[:npart], sm[:npart], mybir.ActivationFunctionType.Exp,
                    bias=nmx[:npart])
                ssum = ew.tile([128, 1], F32, name="ssum", tag="red")
                nc.vector.tensor_reduce(
                    ssum[:npart], ex[:npart], mybir.AxisListType.X,
                    mybir.AluOpType.add)
                rs = ew.tile([128, 1], F32, name="rs", tag="red")
                nc.vector.reciprocal(rs[:npart], ssum[:npart])
                ab = ew.tile([128, S], BF16, name="ab", tag="abf")
                nc.vector.tensor_scalar_mul(ab[:npart], ex[:npart], rs[:npart])
                a_bf.append(ab)
                if _DEBUG:
                    nc.sync.dma_start(
                        dbg["abf"].ap()[ti * 128:ti * 128 + npart, :], ab[:npart])

            # A^T (s'-major) via PE transpose; 2 tiles covering s' 0:128, 128:160
            at = [cp.tile([128, S], BF16, name=f"at{i}", tag=f"at{i}")
                  for i in range(2)]
            blocks = [  # (src tile idx, src col slice, dst tile idx, dst col off)
                (0, 0, 128, 0, 0),
                (1, 0, 128, 0, 128),
                (0, 128, S, 1, 0),
                (1, 128, S, 1, 128),
            ]
            for (sti, c0, c1, dti, dc) in blocks:
                src = a_bf[sti]
                np_src = 128 if sti == 0 else S - 128
                w = c1 - c0
                pt = ps.tile([128, S], BF16, name="pt", tag="pg")
                nc.tensor.transpose(
                    pt[0:w, 0:np_src], src[0:np_src, c0:c1],
                    ident[0:np_src, 0:np_src])
                nc.vector.tensor_copy(
                    at[dti][0:w, dc:dc + np_src], pt[0:w, 0:np_src])

            # hf token-major [S, 512] as 2 partition tiles
            hft = [cp.tile([128, 4, 128], BF16, name=f"hft{i}", tag=f"hft{i}")
                   for i in range(2)]
            for k in range(4):
                pt = ps.tile([128, S], BF16, name="pt2", tag="pg")
                nc.tensor.transpose(
                    pt[0:128, 0:128], hf[k][:, 0:128], ident[:])
                nc.vector.tensor_copy(hft[0][:, k, :], pt[0:128, 0:128])
                pt = ps.tile([128, S], BF16, name="pt3", tag="pg")
                nc.tensor.transpose(
                    pt[0:S - 128, 0:128], hf[k][:, 128:S], ident[:])
                nc.vector.tensor_copy(
                    hft[1][0:S - 128, k, :], pt[0:S - 128, 0:128])

            # att_hist^T [512, S] = hf^T(feature-major result) : contract over s'
            att_b = []
            for m in range(4):
                pa = ps.tile([128, S], F32, name="pa", tag="pg")
                nc.tensor.matmul(pa[:, 0:S], lhsT=hft[0][:, m, :], rhs=at[0][:],
                                 start=True, stop=False)
                nc.tensor.matmul(pa[:, 0:S], lhsT=hft[1][0:S - 128, m, :],
                                 rhs=at[1][0:S - 128, :],
                                 start=False, stop=True)
                ab2 = ew.tile([128, S], BF16, name="ab2", tag="abf")
                nc.vector.tensor_copy(ab2[:], pa[:, 0:S])
                att_b.append(ab2)
                if _DEBUG:
                    nc.sync.dma_start(
                        dbg["att"].ap()[m * 128:(m + 1) * 128, :], ab2[:])

            # out = Q + tanh(att @ W2.T + b2), feature-major [512, S]
            for m in range(4):
                po = ps.tile([128, S], F32, name="po", tag="pg")
                for k in range(4):
                    nc.tensor.matmul(
                        po[:, 0:S],
                        lhsT=w2_sb[:, k, m * 128:(m + 1) * 128],
                        rhs=att_b[k][:],
                        start=(k == 0), stop=(k == 3))
                th = ew.tile([128, S], F32, name="th", tag="ew")
                nc.scalar.activation(
                    th[:], po[:, 0:S], mybir.ActivationFunctionType.Tanh,
                    bias=b2_sb[:, m:m + 1])
                om = op.tile([128, S], F32, name="om", tag="om")
                nc.vector.tensor_add(om[:], th[:], qt_f[m][:])
                nc.sync.dma_start(out_d.ap()[m * 128:(m + 1) * 128, :], om[:])

    nc.compile()
    return nc


def _prep_shared(inp):
    f32 = np.float32
    emb = np.asarray(inp["emb"], f32)
    embp = np.zeros((VOCAB, EPAD), NP_BF16)
    embp[:, :EMB] = emb.astype(NP_BF16)
    embp[0, :] = 0

    perm = _gate_perm()

    def fuse_w(wih, whh):
        w = np.zeros((KQ, G4), f32)
        w[0:EMB, :] = np.asarray(wih, f32).T
        w[EPAD:KQ, :] = np.asarray(whh, f32).T
        return np.ascontiguousarray(w[:, perm]).astype(NP_BF16)

    def fuse_b(bih, bhh):
        bsum = (np.asarray(bih, f32) + np.asarray(bhh, f32))[perm]
        return np.ascontiguousarray(bsum.reshape(NMT, 128).T)

    W1 = np.asarray(inp["W1"], f32)
    shared = {
        "embp": embp,
        "wq": fuse_w(inp["Wih_q"], inp["Whh_q"]),
        "wf": fuse_w(inp["Wih_f"], inp["Whh_f"]),
        "bq": fuse_b(inp["bih_q"], inp["bhh_q"]),
        "bf": fuse_b(inp["bih_f"], inp["bhh_f"]),
        "w1i": np.ascontiguousarray(W1[:, :IMG].T).astype(NP_BF16),
        "w1h": np.ascontiguousarray(W1[:, IMG:].T).astype(NP_BF16),
        "b1": np.ascontiguousarray(
            np.asarray(inp["b1"], f32).reshape(4, 128).T),
        "w2": np.ascontiguousarray(np.asarray(inp["W2"], f32).T).astype(NP_BF16),
        "b2": np.ascontiguousarray(
            np.asarray(inp["b2"], f32).reshape(4, 128).T),
    }
    n = np.arange(S)
    mask = np.where(
        (n[:, None] // R == n[None, :] // R) & (n[None, :] % R <= n[:, None] % R),
        np.float32(0.0), np.float32(NEG))
    shared["mask"] = np.ascontiguousarray(mask.astype(f32))
    return shared


def _prep_core(inp, core):
    sl = slice(core * BS, (core + 1) * BS)

    def tok_idx(arr, L, nblk):
        a = np.asarray(arr[sl], np.int64).reshape(S, L).T  # [L, S] t-major
        return np.ascontiguousarray(
            a.reshape(nblk, 128).T.astype(np.int32))

    img = np.asarray(inp["img_features"], np.float32)[sl]          # [16, 4096]
    img_rep = np.repeat(img, R, axis=0).T                          # [4096, 160]
    return {
        "idxq": tok_idx(inp["questions"], LQ, LQ * S // 128),
        "idxf": tok_idx(inp["history"], LH, LH * S // 128),
        "imgrep": np.ascontiguousarray(img_rep).astype(NP_BF16),
    }


def kernel(**inputs) -> np.ndarray:
    global _STATE
    if _STATE is None:
        _STATE = _build_program()
    nc = _STATE

    shared = _prep_shared(inputs)
    in_maps = []
    for c in range(N_CORES):
        m = dict(shared)
        m.update(_prep_core(inputs, c))
        in_maps.append(m)

    res = run_bass_kernel_spmd(nc, in_maps, core_ids=list(range(N_CORES)))
    outs = []
    for c in range(N_CORES):
        o = np.asarray(res.results[c]["out"], np.float32)   # [512, 160]
        outs.append(o.T.reshape(BS, R, HID))
    return np.concatenate(outs, axis=0)                      # [128, 10, 512]
